# revision 1
# baseline (speedup 1.0000x reference)
"""Trainium2 Bass kernel for nn_MixLinear_GEMM (int4-dequant -> dynamic fp8 GEMM + outlier correction).

Self-contained: kernel(**inputs) takes full inputs, shards across 8 NeuronCores
(tensor-parallel along out_features N), runs one SPMD Bass kernel with
collectives (AllReduce for global maxes, chunked AllGather for fp8-quantized
x^T), and returns the full [M, N] float32 output.

Math notes:
 - reference quantizes to OCP float8_e4m3fn (max 448). TRN2's fp8e4 has max 240,
   so we quantize v/2 instead (max 224) and fold the 2x2 into the output scale.
   For this problem's data every nonzero |v| is far above the subnormal region,
   so the halved TRN rounding is bit-identical to e4m3fn rounding of v.
 - x is quantized to the fp8 grid BEFORE the PE-based transpose: the PE reads
   fp32 as FP22 (truncates mantissa), which would perturb roundings, but
   fp8-grid values pass through FP22 exactly.
 - y = (Xq@Wq^T)*(sx*sw) + bias + x[:,ind]@wc^T  is computed as
   y = psum_main * s4 + ycorr,   s4 = 4*sx*sw
   where psum_main = sum_k (Xq/2)(Wq/2)  (fp8 DoubleRow matmuls) and
   ycorr = xg_aug @ wct_aug^T (f32 matmuls, bias folded in as an extra
   all-ones column) is computed up front into DRAM while the maxes are being
   reduced, to keep the TensorEngine busy during the prologue.
"""
import sys

if "/opt/trn_rl_repo" not in sys.path:
    sys.path.insert(0, "/opt/trn_rl_repo")

import numpy as np

import concourse.bass as bass
import concourse.mybir as mybir
import concourse.tile as tile
from concourse import bacc, bass_isa
from concourse.bass_utils import run_bass_kernel_spmd
from concourse.masks import make_identity

F32 = mybir.dt.float32
I32 = mybir.dt.int32
U8 = mybir.dt.uint8
FP8 = mybir.dt.float8e4
ALU = mybir.AluOpType
AXL = mybir.AxisListType

CORES = 8
GROUP = 128
FP8_HALF_MAX = 224.0  # TRN fp8e4 max is 240; reference e4m3fn max is 448


def build_kernel(M=4096, K=8192, N=8192, CAUG=384):
    """Build the SPMD Bass graph (one graph, runs identically on all 8 cores)."""
    NL = N // CORES          # local out_features
    MSL = M // CORES         # local x row-slice
    KP = K // 128            # number of 128-wide k' chunks
    NWIN = KP // 8           # qwt row windows of 128 (each yields 8 planes)
    MT = MSL // 128          # local m-subtiles
    MB = M // 128            # global m-subtiles
    NB = max(1, NL // 512)   # psum banks per m-tile
    NBW = min(NL, 512)       # psum bank width
    K4 = min(K, 2048)        # x staging tile width
    WPK4 = K4 // 1024        # windows per x staging tile
    NXT = K // K4            # x staging tiles per m-row-tile
    NQ = CAUG // 128
    assert KP % 8 == 0 and MSL % 128 == 0 and NL % NBW == 0 and CAUG % 128 == 0

    nc = bacc.Bacc("TRN2", target_bir_lowering=False, debug=False, num_devices=CORES)

    xs = nc.declare_dram_parameter("xs", [MSL, K], F32, isOutput=False)
    qwt = nc.declare_dram_parameter("qwt", [K // 8, NL], I32, isOutput=False)
    sct = nc.declare_dram_parameter("sct", [K // GROUP, NL], F32, isOutput=False)
    xgt = nc.declare_dram_parameter("xgt", [CAUG, M], F32, isOutput=False)
    wct = nc.declare_dram_parameter("wct", [CAUG, NL], F32, isOutput=False)
    y = nc.declare_dram_parameter("y", [M, NL], F32, isOutput=True)

    with tile.TileContext(nc) as tc:
        with (
            tc.tile_pool(name="const", bufs=1) as constp,
            tc.tile_pool(name="wt", bufs=1) as wtp,
            tc.tile_pool(name="stream", bufs=2) as streamp,
            tc.tile_pool(name="xa", bufs=2) as xap,
            tc.tile_pool(name="xt", bufs=2) as xtp,
            tc.tile_pool(name="ysb", bufs=2) as ysbp,
            tc.tile_pool(name="ycb", bufs=2) as ycbp,
            tc.tile_pool(name="xgc", bufs=4) as xgcp,
            tc.tile_pool(name="psum_t", bufs=2, space="PSUM") as psumt,
            tc.tile_pool(name="psum_mm", bufs=4, space="PSUM") as psummm,
            tc.tile_pool(name="dram", bufs=1, space="DRAM") as dram,
        ):
            ident = constp.tile([128, 128], F32, tag="ident")
            make_identity(nc, ident[:])

            # persistent accumulators / scalars
            xmax_cols = constp.tile([128, MT * NXT], F32, tag="xmax")
            wmax_cols = constp.tile([128, NWIN], F32, tag="wmax")
            gmax_sb = constp.tile([128, 2], F32, tag="gmax")
            rx = constp.tile([128, 1], F32, tag="rx")
            rw = constp.tile([128, 1], F32, tag="rw")
            s4 = constp.tile([128, 1], F32, tag="s4")
            tmp1 = constp.tile([128, 1], F32, tag="tmp1")
            tmp2 = constp.tile([128, 1], F32, tag="tmp2")
            neg8 = constp.tile([128, 1], F32, tag="neg8")
            nc.vector.memset(neg8[:], -8.0)
            lmax2 = constp.tile([128, 2], F32, tag="lmax2")
            lred = constp.tile([128, 2], F32, tag="lred")

            # -------- phase A: local max |W| and max |x| ----------------------
            def load_window_planes(w, engine):
                """DMA window w of qwt, unpack to lo/hi nibble planes (packed int32)."""
                qa = streamp.tile([128, NL], I32, tag="qa")
                nc.sync.dma_start(out=qa[:], in_=qwt[w * 128:(w + 1) * 128, :])
                hi = streamp.tile([128, NL], I32, tag="hi")
                engine.tensor_scalar(hi[:], qa[:], 4, None, ALU.logical_shift_right)
                engine.tensor_scalar(hi[:], hi[:], 0x0F0F0F0F, None, ALU.bitwise_and)
                engine.tensor_scalar(qa[:], qa[:], 0x0F0F0F0F, None, ALU.bitwise_and)
                return qa, hi

            def load_srep(w):
                """Scale rows for window w, replicated 16x across partitions."""
                srep = streamp.tile([128, NL], F32, tag="srep")
                for g in range(8):
                    nc.sync.dma_start(
                        out=srep[g * 16:(g + 1) * 16, :],
                        in_=sct[w * 8 + g:w * 8 + g + 1, :].broadcast_to([16, NL]),
                    )
                return srep

            def plane(lo, hi, j):
                src = lo if j % 2 == 0 else hi
                b = j // 2
                return src[:].bitcast(U8)[:, b::4]

            for w in range(NWIN):
                lo, hi = load_window_planes(w, nc.vector)
                srep = load_srep(w)
                dmaxs = []
                for half in range(2):
                    dmax_h = streamp.tile([128, NL], F32, tag=f"dmax{half}")
                    dmaxs.append(dmax_h)
                    for jj in range(4):
                        j = half * 4 + jj
                        if jj == 0:
                            nc.scalar.activation(
                                out=dmax_h[:], in_=plane(lo, hi, j),
                                func=mybir.ActivationFunctionType.Abs, bias=neg8[:], scale=1.0,
                            )
                        else:
                            dev = streamp.tile([128, NL], F32, tag="dev")
                            nc.scalar.activation(
                                out=dev[:], in_=plane(lo, hi, j),
                                func=mybir.ActivationFunctionType.Abs, bias=neg8[:], scale=1.0,
                            )
                            nc.vector.tensor_tensor(dmax_h[:], dmax_h[:], dev[:], ALU.max)
                nc.vector.tensor_tensor(dmaxs[0][:], dmaxs[0][:], dmaxs[1][:], ALU.max)
                nc.vector.tensor_tensor(dmaxs[0][:], dmaxs[0][:], srep[:], ALU.mult)
                nc.vector.tensor_reduce(
                    out=wmax_cols[:, w:w + 1], in_=dmaxs[0][:],
                    axis=AXL.X, op=ALU.max, apply_absolute_value=True,
                )

            for mt in range(MT):
                for h in range(NXT):
                    xa = xap.tile([128, K4], F32, tag="xa")
                    nc.sync.dma_start(
                        out=xa[:], in_=xs[mt * 128:(mt + 1) * 128, h * K4:(h + 1) * K4]
                    )
                    col = mt * NXT + h
                    nc.vector.tensor_reduce(
                        out=xmax_cols[:, col:col + 1], in_=xa[:],
                        axis=AXL.X, op=ALU.max, apply_absolute_value=True,
                    )

            # -------- correction GEMM prefill: ycorr = xg_aug @ wct_aug^T ----
            # Runs first so the TensorEngine has work while DVE/ACT reduce the
            # maxes. Unscaled f32; added to the scaled main psum in the
            # epilogue. Bias rides along as the all-ones column of xg_aug.
            wct_sb = []
            for q in range(NQ):
                t = constp.tile([128, NL], F32, tag=f"wct{q}")
                nc.gpsimd.dma_start(out=t[:], in_=wct[q * 128:(q + 1) * 128, :])
                wct_sb.append(t)
            ycorr = dram.tile([M, NL], F32, tag="ycorr")
            for b in range(MB):
                xgc = []
                for q in range(NQ):
                    t = xgcp.tile([128, 128], F32, tag="xgc")
                    nc.gpsimd.dma_start(
                        out=t[:], in_=xgt[q * 128:(q + 1) * 128, b * 128:(b + 1) * 128]
                    )
                    xgc.append(t)
                yc_sb = ycbp.tile([128, NL], F32, tag="ycs")
                pscs = []
                for _nb in range(NB):
                    psc_nb = psumt.tile([128, NBW], F32, tag="big")
                    pscs.append(psc_nb)
                for q in range(NQ):
                    for nb in range(NB):
                        nc.tensor.matmul(
                            pscs[nb][:], lhsT=xgc[q][:],
                            rhs=wct_sb[q][:, nb * NBW:(nb + 1) * NBW],
                            start=(q == 0), stop=(q == NQ - 1),
                        )
                for nb in range(NB):
                    nc.scalar.copy(out=yc_sb[:, nb * NBW:(nb + 1) * NBW], in_=pscs[nb][:])
                nc.gpsimd.dma_start(out=ycorr[b * 128:(b + 1) * 128, :], in_=yc_sb[:])

            # -------- AllReduce(max) of (gx, gw), derived scales --------------
            nc.vector.tensor_reduce(
                out=lmax2[:, 0:1], in_=xmax_cols[:], axis=AXL.X,
                op=ALU.max, apply_absolute_value=True,
            )
            nc.vector.tensor_reduce(
                out=lmax2[:, 1:2], in_=wmax_cols[:], axis=AXL.X,
                op=ALU.max, apply_absolute_value=True,
            )
            nc.gpsimd.partition_all_reduce(lred[:], lmax2[:], 128, bass_isa.ReduceOp.max)
            ar_in = dram.tile([1, 2], F32, tag="ar_in")
            ar_out = dram.tile([1, 2], F32, tag="ar_out")
            nc.sync.dma_start(out=ar_in[:], in_=lred[0:1, :])
            nc.gpsimd.collective_compute(
                "AllReduce", ALU.max,
                replica_groups=[list(range(CORES))],
                ins=[ar_in[:].opt()], outs=[ar_out[:].opt()],
            )
            g1 = constp.tile([1, 2], F32, tag="g1")
            nc.sync.dma_start(out=g1[:], in_=ar_out[:])
            nc.gpsimd.partition_broadcast(gmax_sb[:], g1[0:1, :], channels=128)

            # rx = 224/gx, rw = 224/gw, s4 = 4*sx*sw = gx*gw/50176
            # (DVE reciprocal is approximate; two Newton steps make it exact to
            #  f32 so the fp8 rounding boundaries match the reference's x/sx.)
            def refined_recip(out, g_ap):
                nc.vector.reciprocal(tmp1[:], g_ap)
                for _ in range(2):
                    nc.vector.tensor_tensor(tmp2[:], g_ap, tmp1[:], ALU.mult)
                    nc.vector.tensor_scalar(tmp2[:], tmp2[:], -1.0, 2.0, ALU.mult, ALU.add)
                    nc.vector.tensor_tensor(tmp1[:], tmp1[:], tmp2[:], ALU.mult)
                nc.vector.tensor_scalar(out, tmp1[:], FP8_HALF_MAX, None, ALU.mult)

            refined_recip(rx[:], gmax_sb[:, 0:1])
            refined_recip(rw[:], gmax_sb[:, 1:2])
            nc.vector.tensor_tensor(s4[:], gmax_sb[:, 0:1], gmax_sb[:, 1:2], ALU.mult)
            nc.vector.tensor_scalar(s4[:], s4[:], 1.0 / 50176.0, None, ALU.mult)

            # -------- phase B-X: quantize + transpose x slice, AllGather ------
            # fp8-quantize first (ACT), upcast back to f32 (exact), then PE
            # transpose: fp8-grid values survive the PE's FP22 read exactly.
            xga = []
            for mt in range(MT):
                xloc = dram.tile([K, 128], FP8, tag=f"xloc{mt}")
                xga_mt = dram.tile([CORES * K, 128], FP8, tag=f"xga{mt}", addr_space="Shared")
                xga.append(xga_mt)
                xt_sb = xap.tile([128, KP, 128], FP8, tag="xt_sb")
                for h in range(NXT):
                    xa = xap.tile([128, K4], F32, tag="xa")
                    nc.sync.dma_start(
                        out=xa[:], in_=xs[mt * 128:(mt + 1) * 128, h * K4:(h + 1) * K4]
                    )
                    xq8 = xap.tile([128, K4], FP8, tag="xq8")
                    nc.scalar.mul(out=xq8[:], in_=xa[:], mul=rx[:])
                    xb = xap.tile([128, K4], F32, tag="xa")
                    nc.scalar.copy(out=xb[:], in_=xq8[:])
                    for wl in range(WPK4):
                        w = h * WPK4 + wl
                        for j in range(8):
                            pt = psumt.tile([128, 128], F32, tag="pt")
                            nc.tensor.transpose(
                                pt[:], xb[:, wl * 1024 + j: (wl + 1) * 1024: 8], ident[:]
                            )
                            nc.scalar.copy(out=xt_sb[:, w * 8 + j, :], in_=pt[:])
                nc.sync.dma_start(
                    out=xloc[:].rearrange("(c p) m -> p c m", p=128), in_=xt_sb[:]
                )
                nc.gpsimd.collective_compute(
                    "AllGather", ALU.bypass,
                    replica_groups=[list(range(CORES))],
                    ins=[xloc[:].opt()], outs=[xga_mt[:].opt()],
                )

            # -------- phase B-W: quantize weights into resident Wt tiles ------
            wt_sb = []
            for w in range(NWIN):
                wt_w = wtp.tile([128, 8, NL], FP8, tag=f"wt{w}")
                wt_sb.append(wt_w)
                lo, hi = load_window_planes(w, nc.vector)
                srep = load_srep(w)
                nc.vector.tensor_scalar(srep[:], srep[:], rw[:], None, ALU.mult)
                for j in range(8):
                    nc.vector.scalar_tensor_tensor(
                        out=wt_w[:, j, :], in0=plane(lo, hi, j), scalar=-8.0,
                        in1=srep[:], op0=ALU.add, op1=ALU.mult,
                    )

            # -------- main GEMM: fp8 DoubleRow, epilogue adds ycorr -----------
            DR = mybir.MatmulPerfMode.DoubleRow
            for mt in range(MT):
                for c in range(CORES):
                    b = c * MT + mt  # global m-tile index
                    xt_g = xtp.tile([128, KP, 128], FP8, tag="xtg")
                    nc.sync.dma_start(
                        out=xt_g[:],
                        in_=xga[mt][c * K:(c + 1) * K, :].rearrange("(c p) m -> p c m", p=128),
                    )
                    ycb = ycbp.tile([128, NL], F32, tag="ycb")
                    nc.sync.dma_start(out=ycb[:], in_=ycorr[b * 128:(b + 1) * 128, :])
                    pss = []
                    for _nb in range(NB):
                        ps_nb = psummm.tile([128, NBW], F32, tag="ps")
                        pss.append(ps_nb)
                    for t_i in range(KP // 2):
                        w, j = (2 * t_i) // 8, (2 * t_i) % 8
                        for nb in range(NB):
                            nc.tensor.matmul(
                                pss[nb][:],
                                lhsT=xt_g[:, 2 * t_i:2 * t_i + 2, :],
                                rhs=wt_sb[w][:, j:j + 2, nb * NBW:(nb + 1) * NBW],
                                start=(t_i == 0), stop=(t_i == KP // 2 - 1),
                                perf_mode=DR,
                            )
                    y_sb = ysbp.tile([128, NL], F32, tag="ysb")
                    for nb in range(NB):
                        sl = slice(nb * NBW, (nb + 1) * NBW)
                        nc.vector.scalar_tensor_tensor(
                            out=y_sb[:, sl], in0=pss[nb][:], scalar=s4[:],
                            in1=ycb[:, sl], op0=ALU.mult, op1=ALU.add,
                        )
                    nc.sync.dma_start(out=y[b * 128:(b + 1) * 128, :], in_=y_sb[:])

    nc.compile()
    return nc


def shard_inputs(x, q_weight, q_scale_col, weight_cache, ind, bias, M, K, N, CAUG):
    NL = N // CORES
    MSL = M // CORES
    FPn = ind.shape[0]
    x = np.asarray(x, np.float32)
    xg = x[:, np.asarray(ind)]
    xgt = np.zeros((CAUG, M), np.float32)
    xgt[:FPn] = xg.T
    xgt[FPn] = 1.0
    in_maps = []
    for c in range(CORES):
        n0 = c * NL
        wct = np.zeros((CAUG, NL), np.float32)
        wct[:FPn] = np.asarray(weight_cache, np.float32)[n0:n0 + NL].T
        wct[FPn] = np.asarray(bias, np.float32)[n0:n0 + NL]
        in_maps.append({
            "xs": np.ascontiguousarray(x[c * MSL:(c + 1) * MSL]),
            "qwt": np.ascontiguousarray(np.asarray(q_weight, np.int32)[n0:n0 + NL].T),
            "sct": np.ascontiguousarray(np.asarray(q_scale_col, np.float32)[n0:n0 + NL].T),
            "xgt": xgt,
            "wct": wct,
        })
    return in_maps


_NC_CACHE = {}


def get_nc(M=4096, K=8192, N=8192, CAUG=384):
    key = (M, K, N, CAUG)
    if key not in _NC_CACHE:
        _NC_CACHE[key] = build_kernel(M, K, N, CAUG)
    return _NC_CACHE[key]


def kernel(x, q_weight, q_scale_col, weight_cache, ind, bias):
    M, K = x.shape
    N = q_weight.shape[0]
    CAUG = 384
    nc = get_nc(M, K, N, CAUG)
    in_maps = shard_inputs(x, q_weight, q_scale_col, weight_cache, ind, bias, M, K, N, CAUG)
    res = run_bass_kernel_spmd(nc, in_maps, core_ids=list(range(CORES)))
    return np.concatenate([res.results[c]["y"] for c in range(CORES)], axis=1)


if __name__ == "__main__":
    nc = build_kernel()
    print("build+compile ok")



# revision 12
# speedup vs baseline: 1.1119x; 1.1119x over previous
"""Trainium2 Bass kernel for nn_MixLinear_GEMM (int4-dequant -> dynamic fp8 GEMM + outlier correction).

Self-contained: kernel(**inputs) takes full inputs, shards across 8 NeuronCores
(tensor-parallel along out_features N), runs one SPMD Bass kernel with
collectives (AllReduce for the global maxes, per-m-tile AllGather of the
fp8-quantized x^T), and returns the full [M, N] float32 output.

Structure (v2):
 - Host passes x already transposed and k-permuted into matmul "plane" order
   (chunk c of 128 k-rows <-> nibble plane (w=c//8, j=c%8)), so the device
   performs no transposes at all: quantize is a single strided ACT pass per
   m-tile straight into the AllGather staging layout.
 - The outlier correction x[:,ind] @ wc^T (+bias) is fused into the main PSUM
   accumulation as bf16 matmuls with wct pre-scaled by 1/s4, so the epilogue
   is just psum*s4 and there is no DRAM round-trip for the correction term.
 - Phase A (global max|x|, max|W|) is spread across ACT/DVE/GPSIMD while the
   HBM reads stream; one 2-float AllReduce(max) then yields the fp8 scales.

Math notes (same as v1):
 - reference quantizes to OCP float8_e4m3fn (max 448). TRN2's fp8e4 has max
   240, so we quantize v/2 instead (max 224) and fold the 2x2 into the output
   scale s4 = 4*sx*sw = gx*gw/224^2.
 - matching the reference's quantization grid bit-for-bit matters: decorrelated
   fp8 rounding noise between our run and the reference would exceed the 2e-2
   gate on its own. Scales are refined with two Newton steps so the rounding
   boundaries match fl(x/sx) to ~1ulp.
"""
import sys

if "/opt/trn_rl_repo" not in sys.path:
    sys.path.insert(0, "/opt/trn_rl_repo")

import numpy as np

import concourse.bass as bass
import concourse.mybir as mybir
import concourse.tile as tile
from concourse import bacc, bass_isa
from concourse.bass_utils import run_bass_kernel_spmd

F32 = mybir.dt.float32
BF16 = mybir.dt.bfloat16
I32 = mybir.dt.int32
U8 = mybir.dt.uint8
FP8 = mybir.dt.float8e4
ALU = mybir.AluOpType
AXL = mybir.AxisListType
AF = mybir.ActivationFunctionType

CORES = 8
GROUP = 128
FP8_HALF_MAX = 224.0  # TRN fp8e4 max is 240; reference e4m3fn max is 448


def build_kernel(M=4096, K=8192, N=8192, CAUG=384):
    """Build the SPMD Bass graph (one graph, runs identically on all 8 cores)."""
    NL = N // CORES          # local out_features (1024)
    MSL = M // CORES         # local x row-slice (512)
    KP = K // 128            # k'-chunks == nibble planes (64)
    NWIN = KP // 8           # weight windows of 128 qwt rows (8)
    MT = MSL // 128          # local m-tiles (4)
    NB = NL // 512           # psum banks per m-tile (2)
    NBW = 512                # psum bank width
    NQ = CAUG // 128         # correction k-chunks (3)
    XA = 1024                # phase-A x staging width (f32 elems / partition)
    NXA = (K * MSL) // (128 * XA)  # phase-A staging loads (32)
    assert KP % 8 == 0 and MSL % 128 == 0 and NL % NBW == 0 and CAUG % 128 == 0

    nc = bacc.Bacc("TRN2", target_bir_lowering=False, debug=False, num_devices=CORES)

    # x^T, k-permuted into plane order on host: row kk = chunk c*128+p holds
    # original k = ((c//8)*128 + p)*8 + (c%8); columns = this core's m-slice.
    xst = nc.declare_dram_parameter("xst", [K, MSL], F32, isOutput=False)
    qwt = nc.declare_dram_parameter("qwt", [K // 8, NL], I32, isOutput=False)
    sct = nc.declare_dram_parameter("sct", [K // GROUP, NL], F32, isOutput=False)
    xgt = nc.declare_dram_parameter("xgt", [CAUG, M], F32, isOutput=False)
    wct = nc.declare_dram_parameter("wct", [CAUG, NL], F32, isOutput=False)
    y = nc.declare_dram_parameter("y", [M, NL], F32, isOutput=True)

    with tile.TileContext(nc) as tc:
        with (
            tc.tile_pool(name="const", bufs=1) as constp,
            tc.tile_pool(name="wt", bufs=1) as wtp,
            tc.tile_pool(name="wstream", bufs=2) as wsp,
            tc.tile_pool(name="xa", bufs=2) as xap,
            tc.tile_pool(name="xb", bufs=1) as xbp,
            tc.tile_pool(name="xt", bufs=2) as xtp,
            tc.tile_pool(name="xg", bufs=2) as xgp,
            tc.tile_pool(name="ysb", bufs=2) as ysbp,
            tc.tile_pool(name="psum_mm", bufs=4, space="PSUM") as psummm,
            tc.tile_pool(name="dram", bufs=1, space="DRAM") as dram,
        ):
            # persistent scalars / accumulators
            xmax_cols = constp.tile([128, KP], F32, tag="xmax")
            wmax_cols = constp.tile([128, NWIN], F32, tag="wmax")
            gmax_sb = constp.tile([128, 2], F32, tag="gmax")
            rx = constp.tile([128, 1], F32, tag="rx")
            rw = constp.tile([128, 1], F32, tag="rw")
            s4 = constp.tile([128, 1], F32, tag="s4")
            ts4 = constp.tile([128, 1], F32, tag="ts4")
            u4 = constp.tile([128, 1], F32, tag="u4")
            tmp1 = constp.tile([128, 1], F32, tag="tmp1")
            tmp2 = constp.tile([128, 1], F32, tag="tmp2")
            neg8 = constp.tile([128, 1], F32, tag="neg8")
            nc.vector.memset(neg8[:], -8.0)
            lmax2 = constp.tile([128, 2], F32, tag="lmax2")
            lred = constp.tile([128, 2], F32, tag="lred")
            wcts_f = constp.tile([128, NQ, NL], F32, tag="wcts_f")
            wcts = constp.tile([128, NQ, NL], BF16, tag="wcts")

            def plane(lo, hi, j):
                src = lo if j % 2 == 0 else hi
                b = j // 2
                return src[:].bitcast(U8)[:, b::4]

            # -------- phase A: local max |x| and max |W| -----------------------
            # x streams on the sync ring; W unpack spreads over ACT (abs),
            # DVE+GPSIMD (max combines), while qwt streams on the ACT ring.
            NCH = XA // MSL  # chunks per staging load
            for i in range(NXA):
                xa = xap.tile([128, NCH, MSL], F32, tag="xa")
                nc.sync.dma_start(
                    out=xa[:],
                    in_=xst[i * 128 * NCH:(i + 1) * 128 * NCH, :]
                    .rearrange("(c p) m -> p c m", p=128),
                )
                nc.vector.tensor_reduce(
                    out=xmax_cols[:, i * NCH:(i + 1) * NCH], in_=xa[:],
                    axis=AXL.X, op=ALU.max, apply_absolute_value=True,
                )

            for w in range(NWIN):
                qa = wsp.tile([128, NL], I32, tag="qa")
                nc.scalar.dma_start(out=qa[:], in_=qwt[w * 128:(w + 1) * 128, :])
                srep = wsp.tile([128, NL], F32, tag="srep")
                for g in range(8):
                    nc.scalar.dma_start(
                        out=srep[g * 16:(g + 1) * 16, :],
                        in_=sct[w * 8 + g:w * 8 + g + 1, :].broadcast_to([16, NL]),
                    )
                hi = wsp.tile([128, NL], I32, tag="hi")
                nc.vector.tensor_scalar(
                    hi[:], qa[:], 4, 0x0F0F0F0F, ALU.logical_shift_right, ALU.bitwise_and
                )
                nc.vector.tensor_scalar(qa[:], qa[:], 0x0F0F0F0F, None, ALU.bitwise_and)
                # |nib-8| per plane on ACT; max-combine on DVE
                d0 = wsp.tile([128, NL], F32, tag="d0", bufs=1)
                for j in range(8):
                    if j == 0:
                        nc.scalar.activation(
                            out=d0[:], in_=plane(qa, hi, j),
                            func=AF.Abs, bias=neg8[:], scale=1.0,
                        )
                    else:
                        dev = wsp.tile([128, NL], F32, tag="dev")
                        nc.scalar.activation(
                            out=dev[:], in_=plane(qa, hi, j),
                            func=AF.Abs, bias=neg8[:], scale=1.0,
                        )
                        nc.vector.tensor_tensor(d0[:], d0[:], dev[:], ALU.max)
                nc.vector.tensor_tensor(d0[:], d0[:], srep[:], ALU.mult)
                nc.vector.tensor_reduce(
                    out=wmax_cols[:, w:w + 1], in_=d0[:],
                    axis=AXL.X, op=ALU.max, apply_absolute_value=True,
                )

            # correction rhs: load wct f32 now; scaled to bf16 after AR
            for q in range(NQ):
                nc.scalar.dma_start(
                    out=wcts_f[:, q, :], in_=wct[q * 128:(q + 1) * 128, :]
                )

            # prestage m-tile 0 of x^T for the post-AR quantize (pure DMA)
            KH = KP // 2  # x^T staging half (chunks)

            def load_xb_half(mt, h):
                xb32 = xbp.tile([128, KH, 128], F32, tag="xb32")
                nc.sync.dma_start(
                    out=xb32[:],
                    in_=xst[h * KH * 128:(h + 1) * KH * 128, mt * 128:(mt + 1) * 128]
                    .rearrange("(c p) m -> p c m", p=128),
                )
                return xb32

            xb32_0 = load_xb_half(0, 0)

            # -------- AllReduce(max) of (gx, gw), derived scales ---------------
            nc.vector.tensor_reduce(
                out=lmax2[:, 0:1], in_=xmax_cols[:], axis=AXL.X,
                op=ALU.max, apply_absolute_value=True,
            )
            nc.vector.tensor_reduce(
                out=lmax2[:, 1:2], in_=wmax_cols[:], axis=AXL.X,
                op=ALU.max, apply_absolute_value=True,
            )
            nc.gpsimd.partition_all_reduce(lred[:], lmax2[:], 128, bass_isa.ReduceOp.max)
            ar_in = dram.tile([1, 2], F32, tag="ar_in")
            ar_out = dram.tile([1, 2], F32, tag="ar_out")
            nc.sync.dma_start(out=ar_in[:], in_=lred[0:1, :])
            nc.gpsimd.collective_compute(
                "AllReduce", ALU.max,
                replica_groups=[list(range(CORES))],
                ins=[ar_in[:].opt()], outs=[ar_out[:].opt()],
            )
            g1 = constp.tile([1, 2], F32, tag="g1")
            nc.sync.dma_start(out=g1[:], in_=ar_out[:])
            nc.gpsimd.partition_broadcast(gmax_sb[:], g1[0:1, :], channels=128)

            # rx = 224/gx, rw = 224/gw, s4 = gx*gw/50176, ts4 ~= 1/s4
            # (DVE reciprocal is approximate; two Newton steps make it exact to
            #  f32 so the fp8 rounding boundaries match the reference's x/sx.)
            def refined_recip(out, g_ap, factor):
                nc.vector.reciprocal(tmp1[:], g_ap)
                for _ in range(2):
                    nc.vector.tensor_tensor(tmp2[:], g_ap, tmp1[:], ALU.mult)
                    nc.vector.tensor_scalar(tmp2[:], tmp2[:], -1.0, 2.0, ALU.mult, ALU.add)
                    nc.vector.tensor_tensor(tmp1[:], tmp1[:], tmp2[:], ALU.mult)
                nc.vector.tensor_scalar(out, tmp1[:], factor, None, ALU.mult)

            refined_recip(rx[:], gmax_sb[:, 0:1], FP8_HALF_MAX)
            refined_recip(rw[:], gmax_sb[:, 1:2], FP8_HALF_MAX)
            nc.vector.tensor_tensor(u4[:], gmax_sb[:, 0:1], gmax_sb[:, 1:2], ALU.mult)
            nc.vector.tensor_scalar(s4[:], u4[:], 1.0 / (FP8_HALF_MAX * FP8_HALF_MAX), None, ALU.mult)
            refined_recip(ts4[:], u4[:], FP8_HALF_MAX * FP8_HALF_MAX)

            # correction rhs scaled by 1/s4, in bf16 (rides the main psum)
            for q in range(NQ):
                nc.vector.tensor_scalar(
                    wcts[:, q, :], wcts_f[:, q, :], ts4[:], None, ALU.mult
                )

            # -------- phase B-X: quantize x^T per m-tile, AllGather ------------
            xga = []

            def bx_tile(mt, xb32_h0):
                xloc = dram.tile([128, KP * 128], FP8, tag=f"xloc{mt}")
                xga_mt = dram.tile(
                    [CORES * 128, KP * 128], FP8, tag=f"xga{mt}", addr_space="Shared"
                )
                xga.append(xga_mt)
                for h in range(2):
                    xb32 = xb32_h0 if (h == 0 and xb32_h0 is not None) else load_xb_half(mt, h)
                    xt_sb = xtp.tile([128, KH, 128], FP8, tag="xt_sb")
                    nc.scalar.mul(out=xt_sb[:], in_=xb32[:], mul=rx[:])
                    nc.sync.dma_start(
                        out=xloc[:, h * KH * 128:(h + 1) * KH * 128]
                        .rearrange("p (c m) -> p c m", m=128),
                        in_=xt_sb[:],
                    )
                nc.gpsimd.collective_compute(
                    "AllGather", ALU.bypass,
                    replica_groups=[list(range(CORES))],
                    ins=[xloc[:].opt()], outs=[xga_mt[:].opt()],
                )

            bx_tile(0, xb32_0)

            # -------- phase B-W: quantize weights into resident Wt tiles -------
            wt_sb = []
            for w in range(NWIN):
                wt_w = wtp.tile([128, 8, NL], FP8, tag=f"wt{w}")
                wt_sb.append(wt_w)
                qa = wsp.tile([128, NL], I32, tag="qa")
                nc.scalar.dma_start(out=qa[:], in_=qwt[w * 128:(w + 1) * 128, :])
                srep = wsp.tile([128, NL], F32, tag="srep")
                for g in range(8):
                    nc.scalar.dma_start(
                        out=srep[g * 16:(g + 1) * 16, :],
                        in_=sct[w * 8 + g:w * 8 + g + 1, :].broadcast_to([16, NL]),
                    )
                hi = wsp.tile([128, NL], I32, tag="hi")
                nc.vector.tensor_scalar(
                    hi[:], qa[:], 4, 0x0F0F0F0F, ALU.logical_shift_right, ALU.bitwise_and
                )
                nc.vector.tensor_scalar(qa[:], qa[:], 0x0F0F0F0F, None, ALU.bitwise_and)
                nc.vector.tensor_scalar(srep[:], srep[:], rw[:], None, ALU.mult)
                for j in range(8):
                    nc.vector.scalar_tensor_tensor(
                        out=wt_w[:, j, :], in0=plane(qa, hi, j), scalar=-8.0,
                        in1=srep[:], op0=ALU.add, op1=ALU.mult,
                    )

            for mt in range(1, MT):
                bx_tile(mt, None)


            # -------- main GEMM: fp8 DoubleRow + fused bf16 correction ---------
            DR = mybir.MatmulPerfMode.DoubleRow
            for mt in range(MT):
                for c in range(CORES):
                    b = c * MT + mt  # global m-tile index
                    xt_g = xtp.tile([128, KP, 128], FP8, tag="xtg")
                    ring = nc.sync if c % 2 == 0 else nc.scalar
                    ring.dma_start(
                        out=xt_g[:],
                        in_=xga[mt][c * 128:(c + 1) * 128, :]
                        .rearrange("p (c m) -> p c m", m=128),
                    )
                    xgq_f = xgp.tile([128, NQ, 128], F32, tag="xgq_f")
                    for q in range(NQ):
                        nc.scalar.dma_start(
                            out=xgq_f[:, q, :],
                            in_=xgt[q * 128:(q + 1) * 128, b * 128:(b + 1) * 128],
                        )
                    xgq = xgp.tile([128, NQ, 128], BF16, tag="xgq")
                    nc.scalar.copy(out=xgq[:], in_=xgq_f[:])
                    pss = []
                    for _nb in range(NB):
                        ps_nb = psummm.tile([128, NBW], F32, tag="ps")
                        pss.append(ps_nb)
                    for t_i in range(KP // 2):
                        w, j = (2 * t_i) // 8, (2 * t_i) % 8
                        for nb in range(NB):
                            nc.tensor.matmul(
                                pss[nb][:],
                                lhsT=xt_g[:, 2 * t_i:2 * t_i + 2, :],
                                rhs=wt_sb[w][:, j:j + 2, nb * NBW:(nb + 1) * NBW],
                                start=(t_i == 0), stop=False,
                                perf_mode=DR,
                            )
                    for q in range(NQ):
                        for nb in range(NB):
                            nc.tensor.matmul(
                                pss[nb][:], lhsT=xgq[:, q, :],
                                rhs=wcts[:, q, nb * NBW:(nb + 1) * NBW],
                                start=False, stop=(q == NQ - 1),
                            )
                    y_sb = ysbp.tile([128, NL], F32, tag="ysb")
                    for nb in range(NB):
                        nc.vector.tensor_scalar(
                            y_sb[:, nb * NBW:(nb + 1) * NBW], pss[nb][:],
                            s4[:], None, ALU.mult,
                        )
                    nc.gpsimd.dma_start(out=y[b * 128:(b + 1) * 128, :], in_=y_sb[:])

    nc.compile()
    return nc


def shard_inputs(x, q_weight, q_scale_col, weight_cache, ind, bias, M, K, N, CAUG):
    NL = N // CORES
    MSL = M // CORES
    FPn = ind.shape[0]
    x = np.asarray(x, np.float32)
    # k-permutation into plane order: device chunk c=kk//128, p=kk%128 holds
    # original k = ((c//8)*128 + p)*8 + (c%8)  (window w=c//8, nibble j=c%8)
    kk = np.arange(K)
    cc_, pp_ = kk // 128, kk % 128
    perm = ((cc_ // 8) * 128 + pp_) * 8 + (cc_ % 8)
    xp = x[:, perm]
    xg = x[:, np.asarray(ind)]
    xgt = np.zeros((CAUG, M), np.float32)
    xgt[:FPn] = xg.T
    xgt[FPn] = 1.0
    in_maps = []
    for c in range(CORES):
        n0 = c * NL
        wct = np.zeros((CAUG, NL), np.float32)
        wct[:FPn] = np.asarray(weight_cache, np.float32)[n0:n0 + NL].T
        wct[FPn] = np.asarray(bias, np.float32)[n0:n0 + NL]
        in_maps.append({
            "xst": np.ascontiguousarray(xp[c * MSL:(c + 1) * MSL, :].T),
            "qwt": np.ascontiguousarray(np.asarray(q_weight, np.int32)[n0:n0 + NL].T),
            "sct": np.ascontiguousarray(np.asarray(q_scale_col, np.float32)[n0:n0 + NL].T),
            "xgt": xgt,
            "wct": wct,
        })
    return in_maps


_NC_CACHE = {}


def get_nc(M=4096, K=8192, N=8192, CAUG=384):
    key = (M, K, N, CAUG)
    if key not in _NC_CACHE:
        _NC_CACHE[key] = build_kernel(M, K, N, CAUG)
    return _NC_CACHE[key]


def kernel(x, q_weight, q_scale_col, weight_cache, ind, bias):
    M, K = x.shape
    N = q_weight.shape[0]
    CAUG = 384
    nc = get_nc(M, K, N, CAUG)
    in_maps = shard_inputs(x, q_weight, q_scale_col, weight_cache, ind, bias, M, K, N, CAUG)
    res = run_bass_kernel_spmd(nc, in_maps, core_ids=list(range(CORES)))
    return np.concatenate([res.results[c]["y"] for c in range(CORES)], axis=1)


if __name__ == "__main__":
    nc = build_kernel()
    print("build+compile ok")
